# revision 6
# baseline (speedup 1.0000x reference)
"""Segment-softmax feature aggregation (segment_reduce) for Trainium2.

Full inputs: x [8, 256, 128, 128] f32, preds [8, 19, 128, 128] f32.
Sharded batch-parallel across 8 NeuronCores (1 batch per core).

Per-core algorithm (B=1, C=256, N=16384 pixels, K=19 classes):
  s[n]   = max_k preds[k, n]              (per-pixel max logit)
  mask   = (preds == s)                   one-hot argmax (no ties in input)
  wm     = mask * exp(s)                  softmax numerator weights
  agg    = sum_n wm[n,:]^T (.) xt[n,:|1]  PE accumulation -> [k, C+1]
                                          (col C = softmax denominator)
  aggN   = agg[:, :C] / max(den, 1e-30)   (cast bf16)
  out    = aggN^T @ mask[k, n]            PE scatter matmul (bf16)

Layout strategy (v2): all transposes are done host-side during shard
packing, so the device does ZERO data-layout matmuls:
  - x arrives pre-transposed/packed as xt [128, NT=128, C+1] bf16 with
    col C preset to 1.0 (the denominator column).  Each agg matmul's
    rhs is a contiguous [128, 257] slice; bf16 halves the HBM read.
  - preds arrives twice: pixel-major [128, NT, K] f32 (segment max is
    a free-axis vector reduce; s broadcasts along free dim for the
    one-hot compare) and quarter-packed class-major [4, K, 4096] f32
    (for the scatter mask, partitions j*32+k).  The compare runs in
    f32 -- bf16 rounding would create argmax ties and corrupt pixels.
  - s reaches class-major layout via one PE transpose + a tiny HBM
    round-trip with DRE partition-replication on the read-back.
  - out is written bf16 [C, N] and upcast on host: every output value
    is exactly a bf16 aggN value selected by a one-hot mask, so this
    loses nothing vs the f32 write.
A burst of identity matmuls at kernel start warms the PE HAM clock
gate (1.2 -> 2.4 GHz) while the first DMA chunks are in flight.
Input x rides the two HWDGE rings (sync/scalar) in 4x 2 MiB chunks;
preds/s ride the gpsimd SWDGE queue; output alternates HWDGE rings
in 8x 1 MiB chunks.
"""

import numpy as np

B, C, H, W, K = 8, 256, 128, 128, 19
N = H * W                  # 16384
TILE = 128                 # pixels per matmul tile
NT = N // TILE             # 128 n-tiles
CP = C + 1                 # rhs cols (feature cols + denominator col)
QF = N // 4                # 4096 quarter size (class-major free dim)
XCH = 32                   # n-tiles per x DMA chunk (4 chunks of ~2 MiB)
NXC = NT // XCH            # 4 x chunks
OCH = 4096                 # out cols per write chunk (1 MiB bf16)
NWARM = 48                 # PE warm-up matmuls (~5 us cold)
NCORES = 8

_CACHE = {}


def _build_nc():
    import concourse.bacc as bacc
    import concourse.tile as tile
    from concourse import mybir

    f32 = mybir.dt.float32
    bf16 = mybir.dt.bfloat16
    Alu = mybir.AluOpType
    Act = mybir.ActivationFunctionType

    nc = bacc.Bacc("TRN2", target_bir_lowering=True)
    xt_d = nc.dram_tensor("xt", [TILE, NT, CP], bf16, kind="ExternalInput")
    pp_d = nc.dram_tensor("predsP", [TILE, NT, K], f32, kind="ExternalInput")
    pq_d = nc.dram_tensor("predsQ", [4, K, QF], f32, kind="ExternalInput")
    e_d = nc.dram_tensor("ident", [128, 128], f32, kind="ExternalInput")
    o_d = nc.dram_tensor("out", [C, N], bf16, kind="ExternalOutput")
    srow_d = nc.dram_tensor("srow", [4, QF], f32, kind="Internal")

    with tile.TileContext(nc) as tc:
        with tc.tile_pool(name="singles", bufs=1) as singles:
            ident = singles.tile([128, 128], f32)
            identB = singles.tile([128, 128], bf16)
            xtc = [singles.tile([128, XCH, CP], bf16, name=f"xtc{c}")
                   for c in range(NXC)]
            predsP = singles.tile([128, NT, K], f32)
            mt = singles.tile([128, NT, K], f32)
            wmA = singles.tile([128, NT, K], bf16)
            s_all = singles.tile([128, NT], f32)
            es = singles.tile([128, NT], f32)
            sT = singles.tile([128, 128], f32)
            predsQ2 = singles.tile([128, QF], f32)
            s_repQ = singles.tile([128, QF], f32)
            maskQ = singles.tile([128, QF], bf16)
            maskQ3 = singles.tile([K, QF], bf16)   # j=3 (PE can't read p96+)
            aggNb = singles.tile([128, C], bf16)   # replicated at 0/32/64
            dclamp = singles.tile([K, 1], f32)
            dinv = singles.tile([K, 1], f32)

            # ---- input DMA: everything on the two HWDGE rings (SWDGE is
            # ~62 GB/s — only the tiny srow write rides gpsimd).  preds
            # first (they gate the weight computation), then x chunks.
            nc.sync.dma_start(out=ident, in_=e_d[:])
            nc.sync.dma_start(out=predsP, in_=pp_d[:])
            for j in range(4):
                nc.scalar.dma_start(
                    out=predsQ2[j * 32:j * 32 + K, :], in_=pq_d[j]
                )
            for c in range(NXC):
                eng = nc.sync if c % 2 == 0 else nc.scalar
                eng.dma_start(
                    out=xtc[c], in_=xt_d[:, c * XCH:(c + 1) * XCH, :]
                )
            nc.vector.tensor_copy(identB, ident)

            with (
                tc.tile_pool(name="psW", bufs=1, space="PSUM") as psWp,
                tc.tile_pool(name="psS", bufs=1, space="PSUM") as psSp,
                tc.tile_pool(name="psAgg", bufs=1, space="PSUM") as psAggp,
            ):
                # PE warm-up: flip the HAM clock gate while DMA runs
                psWarm = psWp.tile([128, 128], f32)
                for _ in range(NWARM):
                    nc.tensor.matmul(
                        psWarm, lhsT=identB, rhs=identB, start=True, stop=True
                    )

                # ---- Phase 1: s, one-hot, weights (pixel-major) --------
                nc.vector.tensor_reduce(
                    s_all, predsP, axis=mybir.AxisListType.X, op=Alu.max
                )
                nc.scalar.activation(es, s_all, Act.Exp)
                for c in range(NXC):
                    tsl = slice(c * XCH, (c + 1) * XCH)
                    nc.vector.tensor_tensor(
                        out=mt[:, tsl, :], in0=predsP[:, tsl, :],
                        in1=s_all[:, tsl].rearrange("p t -> p t ()")
                        .broadcast_to([128, XCH, K]),
                        op=Alu.is_equal,
                    )
                    nc.vector.tensor_tensor(
                        out=wmA[:, tsl, :], in0=mt[:, tsl, :],
                        in1=es[:, tsl].rearrange("p t -> p t ()")
                        .broadcast_to([128, XCH, K]),
                        op=Alu.mult,
                    )

                # s to class-major: PE transpose + HBM round-trip with
                # DRE partition-replication on the read-back
                psS = psSp.tile([128, 128], f32)
                nc.tensor.transpose(psS, s_all, ident)
                nc.vector.tensor_copy(sT, psS)
                nc.gpsimd.dma_start(
                    out=srow_d.rearrange("j (t p) -> (j t) p", p=TILE),
                    in_=sT,
                )
                for j in range(4):
                    eng = nc.sync if j % 2 == 0 else nc.scalar
                    eng.dma_start(
                        out=s_repQ[j * 32:j * 32 + K, :],
                        in_=srow_d[j:j + 1, :].broadcast_to([K, QF]),
                    )
                # scatter masks (class-major)
                nc.vector.tensor_tensor(
                    out=maskQ[0:115, :], in0=predsQ2[0:115, :],
                    in1=s_repQ[0:115, :], op=Alu.is_equal,
                )
                nc.vector.tensor_tensor(
                    out=maskQ3, in0=predsQ2[96:96 + K, :],
                    in1=s_repQ[96:96 + K, :], op=Alu.is_equal,
                )

                # ---- Phase 2: agg accumulation over all pixel tiles ----
                psAgg = psAggp.tile([K, CP], f32)
                for t in range(NT):
                    c, tt = t // XCH, t % XCH
                    nc.tensor.matmul(
                        psAgg, lhsT=wmA[:, t, :], rhs=xtc[c][:, tt, :],
                        start=(t == 0), stop=(t == NT - 1),
                    )

                # ---- Phase 3: normalize ----
                nc.vector.tensor_scalar(
                    dclamp, psAgg[:, C:C + 1], 1e-30, None, Alu.max
                )
                nc.vector.reciprocal(dinv, dclamp)
                for j in range(3):
                    nc.vector.tensor_scalar(
                        aggNb[j * 32:j * 32 + K, :], psAgg[:, 0:C], dinv,
                        None, Alu.mult,
                    )

            # ---- Phase 4: scatter out = aggN^T @ mask ------------------
            with (
                tc.tile_pool(name="psO", bufs=3, space="PSUM") as psOp,
                tc.tile_pool(name="ost", bufs=3) as ostp,
            ):
                for q in range(N // OCH):          # 4 quarters (OCH == QF)
                    j = q
                    jb = 0 if j == 3 else j * 32
                    for h in range(2):             # feature-row halves
                        ost = ostp.tile([128, OCH], bf16, name="ost")
                        for m2 in range(OCH // 1024):
                            psO = psOp.tile([128, 1024], f32, name="psO")
                            for v in range(2):
                                fs = m2 * 1024 + v * 512
                                rhs = (
                                    maskQ3[:, fs:fs + 512] if j == 3
                                    else maskQ[jb:jb + K, fs:fs + 512]
                                )
                                nc.tensor.matmul(
                                    psO[:, v * 512:(v + 1) * 512],
                                    lhsT=aggNb[jb:jb + K,
                                               h * 128:(h + 1) * 128],
                                    rhs=rhs, start=True, stop=True,
                                )
                            if m2 % 2 == 0:
                                nc.vector.tensor_copy(
                                    ost[:, m2 * 1024:(m2 + 1) * 1024], psO
                                )
                            else:
                                nc.scalar.copy(
                                    ost[:, m2 * 1024:(m2 + 1) * 1024], psO
                                )
                        eng = nc.sync if (q + h) % 2 == 0 else nc.scalar
                        eng.dma_start(
                            out=o_d[h * 128:(h + 1) * 128,
                                    q * OCH:(q + 1) * OCH],
                            in_=ost,
                        )

    nc.compile()
    return nc


def _get_nc():
    if "nc" not in _CACHE:
        _CACHE["nc"] = _build_nc()
    return _CACHE["nc"]


def build_in_maps(x, preds):
    """Host-side shard packing: per-core layouts (see module docstring)."""
    import ml_dtypes

    bf = ml_dtypes.bfloat16
    x = np.asarray(x, dtype=np.float32)
    preds = np.asarray(preds, dtype=np.float32)
    ident = np.eye(128, dtype=np.float32)
    in_maps = []
    for b in range(NCORES):
        xt = np.empty((TILE, NT, CP), dtype=bf)
        # [C, NT, TILE] -> [TILE(p), NT(t), C]
        xt[:, :, :C] = x[b].reshape(C, NT, TILE).transpose(2, 1, 0).astype(bf)
        xt[:, :, C] = np.asarray(1.0, dtype=bf)
        pp = np.ascontiguousarray(
            preds[b].reshape(K, NT, TILE).transpose(2, 1, 0)
        )                                            # [p, t, k] f32
        pq = np.ascontiguousarray(
            preds[b].reshape(K, 4, QF).transpose(1, 0, 2)
        )                                            # [j, k, r] f32
        in_maps.append({"xt": xt, "predsP": pp, "predsQ": pq, "ident": ident})
    return in_maps


def kernel(x, preds):
    from concourse.bass_utils import run_bass_kernel_spmd

    nc = _get_nc()
    in_maps = build_in_maps(x, preds)
    res = run_bass_kernel_spmd(nc, in_maps, list(range(NCORES)))
    out = np.stack(
        [
            np.asarray(res.results[b]["out"]).astype(np.float32).reshape(C, H, W)
            for b in range(NCORES)
        ]
    )
    return out


# revision 20
# speedup vs baseline: 1.5664x; 1.5664x over previous
"""Segment-softmax feature aggregation (segment_reduce) for Trainium2.

HW-verified v2: host-side transpose/pack, bf16 compute, 129.6 us PASS
(rel err 0.00287).  Kept as fallback.
"""

import numpy as np

B, C, H, W, K = 8, 256, 128, 128, 19
N = H * W
TILE = 128
NT = N // TILE
CP = C + 1
QF = N // 4
XCH = 32
NXC = NT // XCH
OCH = 4096
NWARM = 40
NCORES = 8

_CACHE = {}


def _build_nc():
    import concourse.bacc as bacc
    import concourse.tile as tile
    from concourse import mybir

    f32 = mybir.dt.float32
    bf16 = mybir.dt.bfloat16
    Alu = mybir.AluOpType
    Act = mybir.ActivationFunctionType

    nc = bacc.Bacc("TRN2", target_bir_lowering=True)
    xt_d = nc.dram_tensor("xt", [TILE, NT, CP], bf16, kind="ExternalInput")
    pp_d = nc.dram_tensor("predsP", [TILE, NT, K], f32, kind="ExternalInput")
    pq_d = nc.dram_tensor("predsQ", [4, K, QF], f32, kind="ExternalInput")
    e_d = nc.dram_tensor("ident", [128, 128], f32, kind="ExternalInput")
    o_d = nc.dram_tensor("out", [C, N], bf16, kind="ExternalOutput")
    srow_d = nc.dram_tensor("srow", [4, QF], f32, kind="Internal")

    with tile.TileContext(nc) as tc:
        with tc.tile_pool(name="singles", bufs=1) as singles:
            ident = singles.tile([128, 128], f32)
            identB = singles.tile([128, 128], bf16)
            xtc = [singles.tile([128, XCH, CP], bf16, name=f"xtc{c}")
                   for c in range(NXC)]
            predsP = singles.tile([128, NT, K], f32)
            mt = singles.tile([128, NT, K], f32)
            wmA = singles.tile([128, NT, K], bf16)
            s_all = singles.tile([128, NT], f32)
            es = singles.tile([128, NT], f32)
            sT = singles.tile([128, 128], f32)
            predsQ2 = singles.tile([128, QF], f32)
            s_repQ = singles.tile([128, QF], f32)
            maskQ = singles.tile([128, QF], bf16)
            maskQ3 = singles.tile([K, QF], bf16)
            aggNb = singles.tile([128, C], bf16)
            dclamp = singles.tile([K, 1], f32)
            dinv = singles.tile([K, 1], f32)

            nc.sync.dma_start(out=ident, in_=e_d[:])
            for c in range(NXC):
                eng = nc.sync if c % 2 == 0 else nc.scalar
                eng.dma_start(
                    out=xtc[c], in_=xt_d[:, c * XCH:(c + 1) * XCH, :]
                )
            nc.gpsimd.dma_start(out=predsP, in_=pp_d[:])
            for j in range(4):
                nc.gpsimd.dma_start(
                    out=predsQ2[j * 32:j * 32 + K, :], in_=pq_d[j]
                )
            nc.vector.tensor_copy(identB, ident)

            with (
                tc.tile_pool(name="psW", bufs=1, space="PSUM") as psWp,
                tc.tile_pool(name="psS", bufs=1, space="PSUM") as psSp,
                tc.tile_pool(name="psAgg", bufs=1, space="PSUM") as psAggp,
            ):
                psWarm = psWp.tile([128, 128], f32)
                for _ in range(NWARM):
                    nc.tensor.matmul(
                        psWarm, lhsT=identB, rhs=identB, start=True, stop=True
                    )

                nc.vector.tensor_reduce(
                    s_all, predsP, axis=mybir.AxisListType.X, op=Alu.max
                )
                nc.scalar.activation(es, s_all, Act.Exp)
                for c in range(NXC):
                    tsl = slice(c * XCH, (c + 1) * XCH)
                    nc.vector.tensor_tensor(
                        out=mt[:, tsl, :], in0=predsP[:, tsl, :],
                        in1=s_all[:, tsl].rearrange("p t -> p t ()")
                        .broadcast_to([128, XCH, K]),
                        op=Alu.is_equal,
                    )
                    nc.vector.tensor_tensor(
                        out=wmA[:, tsl, :], in0=mt[:, tsl, :],
                        in1=es[:, tsl].rearrange("p t -> p t ()")
                        .broadcast_to([128, XCH, K]),
                        op=Alu.mult,
                    )

                psS = psSp.tile([128, 128], f32)
                nc.tensor.transpose(psS, s_all, ident)
                nc.vector.tensor_copy(sT, psS)
                nc.gpsimd.dma_start(
                    out=srow_d.rearrange("j (t p) -> (j t) p", p=TILE),
                    in_=sT,
                )
                for j in range(4):
                    nc.gpsimd.dma_start(
                        out=s_repQ[j * 32:j * 32 + K, :],
                        in_=srow_d[j:j + 1, :].broadcast_to([K, QF]),
                    )
                nc.vector.tensor_tensor(
                    out=maskQ[0:115, :], in0=predsQ2[0:115, :],
                    in1=s_repQ[0:115, :], op=Alu.is_equal,
                )
                nc.vector.tensor_tensor(
                    out=maskQ3, in0=predsQ2[96:96 + K, :],
                    in1=s_repQ[96:96 + K, :], op=Alu.is_equal,
                )

                psAgg = psAggp.tile([K, CP], f32)
                for t in range(NT):
                    c, tt = t // XCH, t % XCH
                    nc.tensor.matmul(
                        psAgg, lhsT=wmA[:, t, :], rhs=xtc[c][:, tt, :],
                        start=(t == 0), stop=(t == NT - 1),
                    )

                nc.vector.tensor_scalar(
                    dclamp, psAgg[:, C:C + 1], 1e-30, None, Alu.max
                )
                nc.vector.reciprocal(dinv, dclamp)
                for j in range(3):
                    nc.vector.tensor_scalar(
                        aggNb[j * 32:j * 32 + K, :], psAgg[:, 0:C], dinv,
                        None, Alu.mult,
                    )

            with (
                tc.tile_pool(name="psO", bufs=3, space="PSUM") as psOp,
                tc.tile_pool(name="ost", bufs=3) as ostp,
            ):
                for q in range(N // OCH):
                    j = q
                    jb = 0 if j == 3 else j * 32
                    for h in range(2):
                        ost = ostp.tile([128, OCH], bf16, name="ost")
                        for m2 in range(OCH // 1024):
                            psO = psOp.tile([128, 1024], f32, name="psO")
                            for v in range(2):
                                fs = m2 * 1024 + v * 512
                                rhs = (
                                    maskQ3[:, fs:fs + 512] if j == 3
                                    else maskQ[jb:jb + K, fs:fs + 512]
                                )
                                nc.tensor.matmul(
                                    psO[:, v * 512:(v + 1) * 512],
                                    lhsT=aggNb[jb:jb + K,
                                               h * 128:(h + 1) * 128],
                                    rhs=rhs, start=True, stop=True,
                                )
                            if m2 % 2 == 0:
                                nc.vector.tensor_copy(
                                    ost[:, m2 * 1024:(m2 + 1) * 1024], psO
                                )
                            else:
                                nc.scalar.copy(
                                    ost[:, m2 * 1024:(m2 + 1) * 1024], psO
                                )
                        eng = nc.sync if (q + h) % 2 == 0 else nc.scalar
                        eng.dma_start(
                            out=o_d[h * 128:(h + 1) * 128,
                                    q * OCH:(q + 1) * OCH],
                            in_=ost,
                        )

    nc.compile()
    return nc


def _get_nc():
    if "nc" not in _CACHE:
        _CACHE["nc"] = _build_nc()
    return _CACHE["nc"]


def build_in_maps(x, preds):
    import ml_dtypes

    bf = ml_dtypes.bfloat16
    x = np.asarray(x, dtype=np.float32)
    preds = np.asarray(preds, dtype=np.float32)
    ident = np.eye(128, dtype=np.float32)
    in_maps = []
    for b in range(NCORES):
        xt = np.empty((TILE, NT, CP), dtype=bf)
        xt[:, :, :C] = x[b].reshape(C, NT, TILE).transpose(2, 1, 0).astype(bf)
        xt[:, :, C] = np.asarray(1.0, dtype=bf)
        pp = np.ascontiguousarray(
            preds[b].reshape(K, NT, TILE).transpose(2, 1, 0)
        )
        pq = np.ascontiguousarray(
            preds[b].reshape(K, 4, QF).transpose(1, 0, 2)
        )
        in_maps.append({"xt": xt, "predsP": pp, "predsQ": pq, "ident": ident})
    return in_maps


def kernel(x, preds):
    from concourse.bass_utils import run_bass_kernel_spmd

    nc = _get_nc()
    in_maps = build_in_maps(x, preds)
    res = run_bass_kernel_spmd(nc, in_maps, list(range(NCORES)))
    out = np.stack(
        [
            np.asarray(res.results[b]["out"]).astype(np.float32).reshape(C, H, W)
            for b in range(NCORES)
        ]
    )
    return out


# revision 21
# speedup vs baseline: 1.6752x; 1.0694x over previous
"""Segment-softmax feature aggregation (segment_reduce) for Trainium2.

HW-verified v2: host-side transpose/pack, bf16 compute, 129.6 us PASS
(rel err 0.00287).  Kept as fallback.
"""

import numpy as np

B, C, H, W, K = 8, 256, 128, 128, 19
N = H * W
TILE = 128
NT = N // TILE
CP = C + 1
QF = N // 4
XCH = 32
NXC = NT // XCH
OCH = 4096
NWARM = 40
NCORES = 8

_CACHE = {}


def _build_nc():
    import concourse.bacc as bacc
    import concourse.tile as tile
    from concourse import mybir

    f32 = mybir.dt.float32
    bf16 = mybir.dt.bfloat16
    Alu = mybir.AluOpType
    Act = mybir.ActivationFunctionType

    nc = bacc.Bacc("TRN2", target_bir_lowering=True)
    xt_d = nc.dram_tensor("xt", [TILE, NT, CP], bf16, kind="ExternalInput")
    pp_d = nc.dram_tensor("predsP", [TILE, NT, K], f32, kind="ExternalInput")
    pq_d = nc.dram_tensor("predsQ", [4, K, QF], f32, kind="ExternalInput")
    e_d = nc.dram_tensor("ident", [128, 128], f32, kind="ExternalInput")
    o_d = nc.dram_tensor("out", [C, N], bf16, kind="ExternalOutput")
    srow_d = nc.dram_tensor("srow", [4, QF], f32, kind="Internal")

    with tile.TileContext(nc) as tc:
        with tc.tile_pool(name="singles", bufs=1) as singles:
            ident = singles.tile([128, 128], f32)
            identB = singles.tile([128, 128], bf16)
            xtc = [singles.tile([128, XCH, CP], bf16, name=f"xtc{c}")
                   for c in range(NXC)]
            predsP = singles.tile([128, NT, K], f32)
            mt = singles.tile([128, NT, K], f32)
            wmA = singles.tile([128, NT, K], bf16)
            s_all = singles.tile([128, NT], f32)
            es = singles.tile([128, NT], f32)
            sT = singles.tile([128, 128], f32)
            predsQ2 = singles.tile([128, QF], f32)
            s_repQ = singles.tile([128, QF], f32)
            maskQ = singles.tile([128, QF], bf16)
            maskQ3 = singles.tile([K, QF], bf16)
            aggNb = singles.tile([128, C], bf16)
            dclamp = singles.tile([K, 1], f32)
            dinv = singles.tile([K, 1], f32)

            # predsP gates the weight chain: put it on the fast HWDGE ring
            # ahead of x (128-partition shape; the narrow predsQ/broadcast
            # transfers stay on gpsimd, where they are slow but off the
            # critical path until phase 4)
            nc.sync.dma_start(out=ident, in_=e_d[:])
            nc.sync.dma_start(out=predsP, in_=pp_d[:])
            for c in range(NXC):
                eng = nc.scalar if c % 2 == 0 else nc.sync
                eng.dma_start(
                    out=xtc[c], in_=xt_d[:, c * XCH:(c + 1) * XCH, :]
                )
            for j in range(4):
                nc.gpsimd.dma_start(
                    out=predsQ2[j * 32:j * 32 + K, :], in_=pq_d[j]
                )
            nc.vector.tensor_copy(identB, ident)

            with (
                tc.tile_pool(name="psW", bufs=1, space="PSUM") as psWp,
                tc.tile_pool(name="psS", bufs=1, space="PSUM") as psSp,
                tc.tile_pool(name="psAgg", bufs=1, space="PSUM") as psAggp,
            ):
                psWarm = psWp.tile([128, 128], f32)
                for _ in range(NWARM):
                    nc.tensor.matmul(
                        psWarm, lhsT=identB, rhs=identB, start=True, stop=True
                    )

                nc.vector.tensor_reduce(
                    s_all, predsP, axis=mybir.AxisListType.X, op=Alu.max
                )
                nc.scalar.activation(es, s_all, Act.Exp)
                for c in range(NXC):
                    tsl = slice(c * XCH, (c + 1) * XCH)
                    nc.vector.tensor_tensor(
                        out=mt[:, tsl, :], in0=predsP[:, tsl, :],
                        in1=s_all[:, tsl].rearrange("p t -> p t ()")
                        .broadcast_to([128, XCH, K]),
                        op=Alu.is_equal,
                    )
                    nc.vector.tensor_tensor(
                        out=wmA[:, tsl, :], in0=mt[:, tsl, :],
                        in1=es[:, tsl].rearrange("p t -> p t ()")
                        .broadcast_to([128, XCH, K]),
                        op=Alu.mult,
                    )

                psS = psSp.tile([128, 128], f32)
                nc.tensor.transpose(psS, s_all, ident)
                nc.vector.tensor_copy(sT, psS)
                nc.gpsimd.dma_start(
                    out=srow_d.rearrange("j (t p) -> (j t) p", p=TILE),
                    in_=sT,
                )
                for j in range(4):
                    nc.gpsimd.dma_start(
                        out=s_repQ[j * 32:j * 32 + K, :],
                        in_=srow_d[j:j + 1, :].broadcast_to([K, QF]),
                    )
                nc.vector.tensor_tensor(
                    out=maskQ[0:115, :], in0=predsQ2[0:115, :],
                    in1=s_repQ[0:115, :], op=Alu.is_equal,
                )
                nc.vector.tensor_tensor(
                    out=maskQ3, in0=predsQ2[96:96 + K, :],
                    in1=s_repQ[96:96 + K, :], op=Alu.is_equal,
                )

                psAgg = psAggp.tile([K, CP], f32)
                for t in range(NT):
                    c, tt = t // XCH, t % XCH
                    nc.tensor.matmul(
                        psAgg, lhsT=wmA[:, t, :], rhs=xtc[c][:, tt, :],
                        start=(t == 0), stop=(t == NT - 1),
                    )

                nc.vector.tensor_scalar(
                    dclamp, psAgg[:, C:C + 1], 1e-30, None, Alu.max
                )
                nc.vector.reciprocal(dinv, dclamp)
                for j in range(3):
                    nc.vector.tensor_scalar(
                        aggNb[j * 32:j * 32 + K, :], psAgg[:, 0:C], dinv,
                        None, Alu.mult,
                    )

            with (
                tc.tile_pool(name="psO", bufs=3, space="PSUM") as psOp,
                tc.tile_pool(name="ost", bufs=3) as ostp,
            ):
                for q in range(N // OCH):
                    j = q
                    jb = 0 if j == 3 else j * 32
                    for h in range(2):
                        ost = ostp.tile([128, OCH], bf16, name="ost")
                        for m2 in range(OCH // 1024):
                            psO = psOp.tile([128, 1024], f32, name="psO")
                            for v in range(2):
                                fs = m2 * 1024 + v * 512
                                rhs = (
                                    maskQ3[:, fs:fs + 512] if j == 3
                                    else maskQ[jb:jb + K, fs:fs + 512]
                                )
                                nc.tensor.matmul(
                                    psO[:, v * 512:(v + 1) * 512],
                                    lhsT=aggNb[jb:jb + K,
                                               h * 128:(h + 1) * 128],
                                    rhs=rhs, start=True, stop=True,
                                )
                            if m2 % 2 == 0:
                                nc.vector.tensor_copy(
                                    ost[:, m2 * 1024:(m2 + 1) * 1024], psO
                                )
                            else:
                                nc.scalar.copy(
                                    ost[:, m2 * 1024:(m2 + 1) * 1024], psO
                                )
                        eng = nc.sync if (q + h) % 2 == 0 else nc.scalar
                        eng.dma_start(
                            out=o_d[h * 128:(h + 1) * 128,
                                    q * OCH:(q + 1) * OCH],
                            in_=ost,
                        )

    nc.compile()
    return nc


def _get_nc():
    if "nc" not in _CACHE:
        _CACHE["nc"] = _build_nc()
    return _CACHE["nc"]


def build_in_maps(x, preds):
    import ml_dtypes

    bf = ml_dtypes.bfloat16
    x = np.asarray(x, dtype=np.float32)
    preds = np.asarray(preds, dtype=np.float32)
    ident = np.eye(128, dtype=np.float32)
    in_maps = []
    for b in range(NCORES):
        xt = np.empty((TILE, NT, CP), dtype=bf)
        xt[:, :, :C] = x[b].reshape(C, NT, TILE).transpose(2, 1, 0).astype(bf)
        xt[:, :, C] = np.asarray(1.0, dtype=bf)
        pp = np.ascontiguousarray(
            preds[b].reshape(K, NT, TILE).transpose(2, 1, 0)
        )
        pq = np.ascontiguousarray(
            preds[b].reshape(K, 4, QF).transpose(1, 0, 2)
        )
        in_maps.append({"xt": xt, "predsP": pp, "predsQ": pq, "ident": ident})
    return in_maps


def kernel(x, preds):
    from concourse.bass_utils import run_bass_kernel_spmd

    nc = _get_nc()
    in_maps = build_in_maps(x, preds)
    res = run_bass_kernel_spmd(nc, in_maps, list(range(NCORES)))
    out = np.stack(
        [
            np.asarray(res.results[b]["out"]).astype(np.float32).reshape(C, H, W)
            for b in range(NCORES)
        ]
    )
    return out


# revision 24
# speedup vs baseline: 1.8855x; 1.1255x over previous
"""Segment-softmax feature aggregation (segment_reduce) for Trainium2.

HW-verified v2: host-side transpose/pack, bf16 compute, 129.6 us PASS
(rel err 0.00287).  Kept as fallback.
"""

import numpy as np

B, C, H, W, K = 8, 256, 128, 128, 19
N = H * W
TILE = 128
NT = N // TILE
CP = C + 1
QF = N // 4
XCH = 32
NXC = NT // XCH
OCH = 4096
NWARM = 40
NCORES = 8

_CACHE = {}


def _build_nc():
    import concourse.bacc as bacc
    import concourse.tile as tile
    from concourse import mybir

    f32 = mybir.dt.float32
    bf16 = mybir.dt.bfloat16
    Alu = mybir.AluOpType
    Act = mybir.ActivationFunctionType

    nc = bacc.Bacc("TRN2", target_bir_lowering=True)
    xt_d = nc.dram_tensor("xt", [TILE, NT, CP], bf16, kind="ExternalInput")
    pp_d = nc.dram_tensor("predsP", [TILE, NT, K], f32, kind="ExternalInput")
    pq_d = nc.dram_tensor("predsQ", [128, QF], f32, kind="ExternalInput")
    e_d = nc.dram_tensor("ident", [128, 128], f32, kind="ExternalInput")
    o_d = nc.dram_tensor("out", [C, N], bf16, kind="ExternalOutput")
    srow_d = nc.dram_tensor("srow", [4, QF], f32, kind="Internal")

    with tile.TileContext(nc) as tc:
        with tc.tile_pool(name="singles", bufs=1) as singles:
            ident = singles.tile([128, 128], f32)
            identB = singles.tile([128, 128], bf16)
            xtc = [singles.tile([128, XCH, CP], bf16, name=f"xtc{c}")
                   for c in range(NXC)]
            predsP = singles.tile([128, NT, K], f32)
            mt = singles.tile([128, NT, K], f32)
            wmA = singles.tile([128, NT, K], bf16)
            s_all = singles.tile([128, NT], f32)
            es = singles.tile([128, NT], f32)
            sT = singles.tile([128, 128], f32)
            predsQ2 = singles.tile([128, QF], f32)
            s_repQ = singles.tile([128, QF], f32)
            maskQ = singles.tile([128, QF], bf16)
            maskQ3 = singles.tile([K, QF], bf16)
            aggNb = singles.tile([128, C], bf16)
            dclamp = singles.tile([K, 1], f32)
            dinv = singles.tile([K, 1], f32)

            # predsP gates the weight chain: put it on the fast HWDGE ring
            # ahead of x (128-partition shape; the narrow predsQ/broadcast
            # transfers stay on gpsimd, where they are slow but off the
            # critical path until phase 4)
            nc.sync.dma_start(out=ident, in_=e_d[:])
            nc.sync.dma_start(out=predsP, in_=pp_d[:])
            for c in range(NXC):
                eng = nc.scalar if c % 2 == 0 else nc.sync
                eng.dma_start(
                    out=xtc[c], in_=xt_d[:, c * XCH:(c + 1) * XCH, :]
                )
            # class-major preds: host-padded to 32 classes/quarter so one
            # 128-partition HWDGE DMA carries it (after x in the FIFO --
            # it is only needed for phase 4); pad rows are -1e30, never
            # equal to s
            nc.scalar.dma_start(out=predsQ2, in_=pq_d[:])
            nc.vector.tensor_copy(identB, ident)

            with (
                tc.tile_pool(name="psW", bufs=1, space="PSUM") as psWp,
                tc.tile_pool(name="psS", bufs=1, space="PSUM") as psSp,
                tc.tile_pool(name="psAgg", bufs=1, space="PSUM") as psAggp,
            ):
                psWarm = psWp.tile([128, 128], f32)
                for _ in range(NWARM):
                    nc.tensor.matmul(
                        psWarm, lhsT=identB, rhs=identB, start=True, stop=True
                    )

                nc.vector.tensor_reduce(
                    s_all, predsP, axis=mybir.AxisListType.X, op=Alu.max
                )
                nc.scalar.activation(es, s_all, Act.Exp)
                for c in range(NXC):
                    tsl = slice(c * XCH, (c + 1) * XCH)
                    nc.vector.tensor_tensor(
                        out=mt[:, tsl, :], in0=predsP[:, tsl, :],
                        in1=s_all[:, tsl].rearrange("p t -> p t ()")
                        .broadcast_to([128, XCH, K]),
                        op=Alu.is_equal,
                    )
                    nc.vector.tensor_tensor(
                        out=wmA[:, tsl, :], in0=mt[:, tsl, :],
                        in1=es[:, tsl].rearrange("p t -> p t ()")
                        .broadcast_to([128, XCH, K]),
                        op=Alu.mult,
                    )

                psS = psSp.tile([128, 128], f32)
                nc.tensor.transpose(psS, s_all, ident)
                nc.vector.tensor_copy(sT, psS)
                nc.gpsimd.dma_start(
                    out=srow_d.rearrange("j (t p) -> (j t) p", p=TILE),
                    in_=sT,
                )
                for j in range(4):
                    nc.gpsimd.dma_start(
                        out=s_repQ[j * 32:j * 32 + K, :],
                        in_=srow_d[j:j + 1, :].broadcast_to([K, QF]),
                    )
                nc.vector.tensor_tensor(
                    out=maskQ[0:115, :], in0=predsQ2[0:115, :],
                    in1=s_repQ[0:115, :], op=Alu.is_equal,
                )
                nc.vector.tensor_tensor(
                    out=maskQ3, in0=predsQ2[96:96 + K, :],
                    in1=s_repQ[96:96 + K, :], op=Alu.is_equal,
                )

                psAgg = psAggp.tile([K, CP], f32)
                for t in range(NT):
                    c, tt = t // XCH, t % XCH
                    nc.tensor.matmul(
                        psAgg, lhsT=wmA[:, t, :], rhs=xtc[c][:, tt, :],
                        start=(t == 0), stop=(t == NT - 1),
                    )

                nc.vector.tensor_scalar(
                    dclamp, psAgg[:, C:C + 1], 1e-30, None, Alu.max
                )
                nc.vector.reciprocal(dinv, dclamp)
                for j in range(3):
                    nc.vector.tensor_scalar(
                        aggNb[j * 32:j * 32 + K, :], psAgg[:, 0:C], dinv,
                        None, Alu.mult,
                    )

            with (
                tc.tile_pool(name="psO", bufs=3, space="PSUM") as psOp,
                tc.tile_pool(name="ost", bufs=3) as ostp,
            ):
                for q in range(N // OCH):
                    j = q
                    jb = 0 if j == 3 else j * 32
                    for h in range(2):
                        ost = ostp.tile([128, OCH], bf16, name="ost")
                        for m2 in range(OCH // 1024):
                            psO = psOp.tile([128, 1024], f32, name="psO")
                            for v in range(2):
                                fs = m2 * 1024 + v * 512
                                rhs = (
                                    maskQ3[:, fs:fs + 512] if j == 3
                                    else maskQ[jb:jb + K, fs:fs + 512]
                                )
                                nc.tensor.matmul(
                                    psO[:, v * 512:(v + 1) * 512],
                                    lhsT=aggNb[jb:jb + K,
                                               h * 128:(h + 1) * 128],
                                    rhs=rhs, start=True, stop=True,
                                )
                            if m2 % 2 == 0:
                                nc.vector.tensor_copy(
                                    ost[:, m2 * 1024:(m2 + 1) * 1024], psO
                                )
                            else:
                                nc.scalar.copy(
                                    ost[:, m2 * 1024:(m2 + 1) * 1024], psO
                                )
                        eng = nc.sync if (q + h) % 2 == 0 else nc.scalar
                        eng.dma_start(
                            out=o_d[h * 128:(h + 1) * 128,
                                    q * OCH:(q + 1) * OCH],
                            in_=ost,
                        )

    nc.compile()
    return nc


def _get_nc():
    if "nc" not in _CACHE:
        _CACHE["nc"] = _build_nc()
    return _CACHE["nc"]


def build_in_maps(x, preds):
    import ml_dtypes

    bf = ml_dtypes.bfloat16
    x = np.asarray(x, dtype=np.float32)
    preds = np.asarray(preds, dtype=np.float32)
    ident = np.eye(128, dtype=np.float32)
    in_maps = []
    for b in range(NCORES):
        xt = np.empty((TILE, NT, CP), dtype=bf)
        xt[:, :, :C] = x[b].reshape(C, NT, TILE).transpose(2, 1, 0).astype(bf)
        xt[:, :, C] = np.asarray(1.0, dtype=bf)
        pp = np.ascontiguousarray(
            preds[b].reshape(K, NT, TILE).transpose(2, 1, 0)
        )
        # class-major, padded to 32 classes/quarter so the DMA spans all
        # 128 partitions; pad rows compare unequal to any s
        pq = np.full((4, 32, QF), -1e30, dtype=np.float32)
        pq[:, :K, :] = preds[b].reshape(K, 4, QF).transpose(1, 0, 2)
        in_maps.append({
            "xt": xt, "predsP": pp,
            "predsQ": pq.reshape(128, QF), "ident": ident,
        })
    return in_maps


def kernel(x, preds):
    from concourse.bass_utils import run_bass_kernel_spmd

    nc = _get_nc()
    in_maps = build_in_maps(x, preds)
    res = run_bass_kernel_spmd(nc, in_maps, list(range(NCORES)))
    out = np.stack(
        [
            np.asarray(res.results[b]["out"]).astype(np.float32).reshape(C, H, W)
            for b in range(NCORES)
        ]
    )
    return out
